# revision 2
# baseline (speedup 1.0000x reference)
"""CoAttention ImageDNS kernel for Trainium2 (8 NeuronCores, Bass/Tile).

Math: the reference computes two additive-attention blocks. In both, the
softmax'd score is  score[b, q, k] = f(q-side)[b, q] + g(k-side)[b, k] + c,
and softmax over k is invariant to the q-dependent (and constant) terms, so
the attention weights are independent of the query index:

  visual_att[b, s, :]  = softmax_r( wB . tanh(W_i1 @ img[b, r]) )
  textual_att[b, i, :] = softmax_j( wD . tanh(W_d2 @ dns[b, j]) )

Hence both outputs are per-batch rank-1 broadcasts:

  att_img_features[b, s, :] = visual_att[b]  @ img[b]   (same for all s)
  att_dns_features[b, i, :] = textual_att[b] @ dns[b]   (same for all i)

W_d1/b_d1/w_att1[:H]/b_att1/W_i2/b_i2/w_att2[:H]/b_att2 cancel entirely.

Sharding: pure data-parallel over batch, 4 batches per core, no collectives.
The device computes the per-batch [H] attention outputs; the host broadcasts
them over the (identical) S query rows, so the kernel writes only B*H values
instead of B*S*H.

Projection matmuls run in fp8e4 with perf_mode=DoubleRow (2 MACs/cell/cycle,
contraction 256 per matmul). The projection weights are pre-scaled by 32 on
the host so their entries (~N(0, 1/1024)) sit in fp8e4's normal range; the
1/32 is folded into the tanh activation's scale. Both projections that
survive the softmax cancellation (W_i1, W_d2) are bias-free, so the scale
factors through cleanly. The attention-weighted sum over the raw features
(stage 2) stays bf16 for accuracy; softmax/normalization is fp32.
"""

import sys
import numpy as np
import ml_dtypes

_BF16 = ml_dtypes.bfloat16
_F8 = ml_dtypes.float8_e4m3fn

for _p in ("/opt/trn_rl_repo", "/root/.axon_site/_ro/trn_rl_repo"):
    if _p not in sys.path:
        sys.path.append(_p)

B, S, R, H = 32, 512, 196, 1024
NCORES = 8
BLOC = B // NCORES          # batches per core
OC = 512                    # output-chunk (one fp32 PSUM bank)
HC = H // 128               # contraction chunks of 128
HC2 = H // 256              # contraction chunks of 256 (fp8 DoubleRow)
USE_FP8 = True
W_SCALE = 32.0              # host pre-scale on fp8 weights; undone in tanh

_CACHE = {}


def _row_chunks(n):
    out, o = [], 0
    while o < n:
        out.append((o, min(128, n - o)))
        o += 128
    return out


def build_nc(use_fp8=USE_FP8):
    from concourse import bacc, mybir
    from concourse import tile

    f32, f16 = mybir.dt.float32, mybir.dt.bfloat16
    f8 = mybir.dt.float8e4
    xdt = f8 if use_fp8 else f16
    Act = mybir.ActivationFunctionType
    Alu = mybir.AluOpType
    DR = mybir.MatmulPerfMode.DoubleRow if use_fp8 else None

    nc = bacc.Bacc("TRN2", target_bir_lowering=False, debug=False)

    RP = 256  # img row count padded to a partition multiple for single-DMA loads
    xt_dns = nc.dram_tensor("xt_dns", [BLOC, HC, 128, S], xdt, kind="ExternalInput")
    xn_dns = nc.dram_tensor("xn_dns", [BLOC, S, H], f16, kind="ExternalInput")
    xt_img = nc.dram_tensor("xt_img", [BLOC, HC, 128, R], xdt, kind="ExternalInput")
    xn_img = nc.dram_tensor("xn_img", [BLOC, RP, H], f16, kind="ExternalInput")
    wt_i1 = nc.dram_tensor("wt_i1", [HC, 128, H], xdt, kind="ExternalInput")
    wt_d2 = nc.dram_tensor("wt_d2", [HC, 128, H], xdt, kind="ExternalInput")
    wrow_b = nc.dram_tensor("wrow_b", [128, H], f16, kind="ExternalInput")
    wrow_d = nc.dram_tensor("wrow_d", [128, H], f16, kind="ExternalInput")
    out_dns = nc.dram_tensor("out_dns", [BLOC, H], f32, kind="ExternalOutput")
    out_img = nc.dram_tensor("out_img", [BLOC, H], f32, kind="ExternalOutput")

    tanh_scale = (1.0 / W_SCALE) if use_fp8 else 1.0

    with tile.TileContext(nc) as tc:
        with (
            tc.tile_pool(name="const", bufs=1) as cpool,
            tc.tile_pool(name="xts", bufs=2) as xtpool,
            tc.tile_pool(name="xns", bufs=2) as xnpool,
            tc.tile_pool(name="work", bufs=3) as wpool,
            tc.tile_pool(name="small", bufs=12) as spool,
            tc.tile_pool(name="outs", bufs=2) as opool,
            tc.tile_pool(name="pp", bufs=3, space="PSUM") as ppool,
            tc.tile_pool(name="ps", bufs=2, space="PSUM") as pstat,
        ):
            # lazy const loads: weight DMAs are interleaved with the first
            # activation loads (per-chunk) at first use, so the first
            # projection group's dependencies land early in the queue
            wt_sb, wrow_sb = {}, {}

            def get_wt(nm):
                if nm not in wt_sb:
                    w = cpool.tile([128, HC * H], xdt, name=f"wt_{nm}_sb")
                    wt_sb[nm] = w
                return wt_sb[nm]

            def load_wt_chunk(nm, hc):
                dram = {"i1": wt_i1, "d2": wt_d2}[nm]
                w = wt_sb[nm]
                nc.sync.dma_start(out=w[:, hc * H:(hc + 1) * H], in_=dram[hc])

            def get_wrow(nm):
                if nm not in wrow_sb:
                    dram = {"b": wrow_b, "d": wrow_d}[nm]
                    w = cpool.tile([128, H], f16, name=f"wrow_{nm}_sb")
                    nc.sync.dma_start(out=w[:, :], in_=dram[:, :])
                    wrow_sb[nm] = w
                return wrow_sb[nm]

            ones_col = cpool.tile([128, 1], f16, name="ones_col")
            nc.vector.memset(ones_col[:, :], 1.0)

            for b in range(BLOC):
                for side in ("img", "dns"):
                    n_rows = R if side == "img" else S
                    xt_d = xt_img if side == "img" else xt_dns
                    xn_d = xn_img if side == "img" else xn_dns
                    wt_name = "i1" if side == "img" else "d2"
                    load_wt = wt_name not in wt_sb
                    wt = get_wt(wt_name)
                    out_d = out_img if side == "img" else out_dns
                    rcs = _row_chunks(n_rows)

                    # -- loads: on a weight's first use, interleave per-hc wt/xt
                    # chunks so the first projection group's deps land first;
                    # afterwards one 3D DMA covers the whole xt tile --
                    xt_t = xtpool.tile([128, HC * n_rows], xdt,
                                       name=f"xt_{side}_{b}", tag=f"xt_{side}")
                    if load_wt:
                        for hc in range(HC):
                            load_wt_chunk(wt_name, hc)
                            nc.sync.dma_start(
                                out=xt_t[:, hc * n_rows:(hc + 1) * n_rows],
                                in_=xt_d[b, hc])
                    else:
                        nc.sync.dma_start(
                            out=xt_t.rearrange("p (hc m) -> p hc m", hc=HC),
                            in_=xt_d[b].rearrange("hc p m -> p hc m"))
                    xt3 = xt_t.rearrange("p (hc m) -> p hc m", hc=HC)
                    wt3 = wt.rearrange("p (hc m) -> p hc m", hc=HC)

                    # -- projection, tanh, weighted o-reduction, exp --
                    # xn / wrow loads are issued after the first proj group so
                    # the projection's own dependencies lead the DMA queues
                    acols = []
                    xn_ts = []
                    wr = None
                    s_ps = pstat.tile([1, 1], f32, name=f"s_{side}_{b}", tag="stat")
                    for ci, (r0, rk) in enumerate(rcs):
                        ps = ppool.tile([128, H], f32, name=f"proj_{side}_{ci}_{b}",
                                        tag="pp")
                        if use_fp8:
                            for c in range(HC2):
                                lhs = xt3[:, 2 * c:2 * c + 2, r0:r0 + rk]
                                for oc in range(2):
                                    nc.tensor.matmul(
                                        ps[0:rk, oc * OC:(oc + 1) * OC],
                                        lhsT=lhs,
                                        rhs=wt3[:, 2 * c:2 * c + 2,
                                                oc * OC:(oc + 1) * OC],
                                        start=(c == 0), stop=(c == HC2 - 1),
                                        perf_mode=DR)
                        else:
                            for hc in range(HC):
                                lhs = xt3[:, hc, r0:r0 + rk]
                                for oc in range(2):
                                    nc.tensor.matmul(
                                        ps[0:rk, oc * OC:(oc + 1) * OC],
                                        lhsT=lhs,
                                        rhs=wt3[:, hc, oc * OC:(oc + 1) * OC],
                                        start=(hc == 0), stop=(hc == HC - 1))
                        if ci == 0:
                            nrc = len(rcs)
                            xn_t = xnpool.tile([128, nrc * H], f16,
                                               name=f"xn_{side}_{b}", tag=f"xn_{side}")
                            nc.sync.dma_start(
                                out=xn_t.rearrange("p (rc n) -> p rc n", rc=nrc),
                                in_=xn_d[b, 0:nrc * 128, :]
                                .rearrange("(rc p) n -> p rc n", p=128))
                            xn_ts = [xn_t[:, cj * H:(cj + 1) * H] for cj in range(nrc)]
                            wr = get_wrow("b" if side == "img" else "d")
                        th = wpool.tile([128, H], f16, name=f"th_{side}_{ci}_{b}", tag="th")
                        nc.scalar.activation(th[0:rk, :], ps[0:rk, :], Act.Tanh,
                                             scale=tanh_scale)
                        scr = wpool.tile([128, H], f16, name=f"scr_{side}_{ci}_{b}",
                                         tag="scr", bufs=2)
                        tcol = spool.tile([128, 1], f32, name=f"tc_{side}_{ci}_{b}", tag="tcol")
                        nc.vector.scalar_tensor_tensor(
                            out=scr[0:rk, :], in0=th[0:rk, :], scalar=1.0,
                            in1=wr[0:rk, :], op0=Alu.mult, op1=Alu.mult,
                            accum_out=tcol[0:rk, :])
                        acol = spool.tile([128, 1], f16, name=f"a_{side}_{ci}_{b}",
                                          tag=f"acol_{side}_{ci}")
                        nc.scalar.activation(acol[0:rk, :], tcol[0:rk, :], Act.Exp)
                        acols.append((acol, rk))
                        nc.tensor.matmul(
                            s_ps[0:1, 0:1], lhsT=acol[0:rk, 0:1], rhs=ones_col[0:rk, 0:1],
                            start=(ci == 0), stop=(ci == len(rcs) - 1))

                    # -- 1/sum, broadcast to 128 partitions (idle GPSIMD) --
                    r_sb = spool.tile([1, 1], f32, name=f"r_{side}_{b}", tag="r")
                    nc.vector.reciprocal(r_sb[0:1, 0:1], s_ps[0:1, 0:1])
                    rb_sb = spool.tile([128, 1], f32, name=f"rbs_{side}_{b}", tag="rb")
                    nc.gpsimd.partition_broadcast(rb_sb[:, 0:1], r_sb[0:1, 0:1])

                    # -- stage 2: out[h] = sum_r a_r x[r, h] (M=128 rows, all
                    # equal; only row 0 is written out) --
                    att_ps = ppool.tile([128, H], f32, name=f"att_{side}_{b}", tag="pp")
                    for h2 in range(2):
                        for ci, (r0, rk) in enumerate(rcs):
                            acol, _ = acols[ci]
                            nc.tensor.matmul(
                                att_ps[:, h2 * OC:(h2 + 1) * OC],
                                lhsT=acol[0:rk, 0:1].to_broadcast((rk, 128)),
                                rhs=xn_ts[ci][0:rk, h2 * OC:(h2 + 1) * OC],
                                start=(ci == 0), stop=(ci == len(rcs) - 1))
                    att_sb = opool.tile([128, H], f32, name=f"attsb_{side}_{b}",
                                        tag=f"att_{side}")
                    for h2 in range(2):
                        nc.scalar.activation(att_sb[:, h2 * OC:(h2 + 1) * OC],
                                             att_ps[:, h2 * OC:(h2 + 1) * OC],
                                             Act.Copy, scale=rb_sb[:, 0:1])
                    nc.sync.dma_start(out=out_d[b:b + 1, :], in_=att_sb[0:1, :])
    nc.compile()
    return nc


def _get_nc():
    if "nc" not in _CACHE:
        _CACHE["nc"] = build_nc()
    return _CACHE["nc"]


def make_in_maps(inputs):
    dns = np.ascontiguousarray(np.asarray(inputs["dns_feature"], dtype=np.float32))
    img = np.ascontiguousarray(np.asarray(inputs["img_features"], dtype=np.float32))
    W_i1 = np.asarray(inputs["W_i1"], dtype=np.float32)
    W_d2 = np.asarray(inputs["W_d2"], dtype=np.float32)
    wB = np.asarray(inputs["w_att1"], dtype=np.float32)[H:]
    wD = np.asarray(inputs["w_att2"], dtype=np.float32)[H:]

    xdt = _F8 if USE_FP8 else _BF16
    wsc = W_SCALE if USE_FP8 else 1.0
    wt_i1 = np.ascontiguousarray(W_i1.T * wsc).reshape(HC, 128, H).astype(xdt)
    wt_d2 = np.ascontiguousarray(W_d2.T * wsc).reshape(HC, 128, H).astype(xdt)
    wrow_b = np.ascontiguousarray(np.broadcast_to(wB, (128, H))).astype(_BF16)
    wrow_d = np.ascontiguousarray(np.broadcast_to(wD, (128, H))).astype(_BF16)

    xt_dns = np.ascontiguousarray(
        dns.transpose(0, 2, 1).reshape(B, HC, 128, S).astype(xdt))
    xt_img = np.ascontiguousarray(
        img.transpose(0, 2, 1).reshape(B, HC, 128, R).astype(xdt))
    xn_dns = dns.astype(_BF16)
    xn_img = np.zeros((B, 256, H), dtype=_BF16)
    xn_img[:, :R, :] = img.astype(_BF16)

    in_maps = []
    for k in range(NCORES):
        sl = slice(k * BLOC, (k + 1) * BLOC)
        in_maps.append({
            "xt_dns": np.ascontiguousarray(xt_dns[sl]),
            "xn_dns": np.ascontiguousarray(xn_dns[sl]),
            "xt_img": np.ascontiguousarray(xt_img[sl]),
            "xn_img": np.ascontiguousarray(xn_img[sl]),
            "wt_i1": wt_i1,
            "wt_d2": wt_d2,
            "wrow_b": wrow_b,
            "wrow_d": wrow_d,
        })
    return in_maps


def kernel(**inputs):
    from concourse.bass_utils import run_bass_kernel_spmd

    nc = _get_nc()
    in_maps = make_in_maps(inputs)
    res = run_bass_kernel_spmd(nc, in_maps, list(range(NCORES))).results
    att_dns = np.concatenate([res[k]["out_dns"] for k in range(NCORES)], axis=0)
    att_img = np.concatenate([res[k]["out_img"] for k in range(NCORES)], axis=0)
    out_dns = np.ascontiguousarray(
        np.broadcast_to(att_dns[:, None, :], (B, S, H)))
    out_img = np.ascontiguousarray(
        np.broadcast_to(att_img[:, None, :], (B, S, H)))
    return out_dns, out_img


# revision 3
# speedup vs baseline: 1.2338x; 1.2338x over previous
"""CoAttention ImageDNS kernel for Trainium2 (8 NeuronCores, Bass/Tile).

Math: the reference computes two additive-attention blocks. In both, the
softmax'd score is  score[b, q, k] = f(q-side)[b, q] + g(k-side)[b, k] + c,
and softmax over k is invariant to the q-dependent (and constant) terms, so
the attention weights are independent of the query index:

  visual_att[b, s, :]  = softmax_r( wB . tanh(W_i1 @ img[b, r]) )
  textual_att[b, i, :] = softmax_j( wD . tanh(W_d2 @ dns[b, j]) )

Hence both outputs are per-batch rank-1 broadcasts:

  att_img_features[b, s, :] = visual_att[b]  @ img[b]   (same for all s)
  att_dns_features[b, i, :] = textual_att[b] @ dns[b]   (same for all i)

W_d1/b_d1/w_att1[:H]/b_att1/W_i2/b_i2/w_att2[:H]/b_att2 cancel entirely.

Sharding: pure data-parallel over batch, 4 batches per core, no collectives.
The device computes the per-batch [H] attention outputs; the host broadcasts
them over the (identical) S query rows, so the kernel writes only B*H values
instead of B*S*H.

Projection matmuls run in fp8e4 with perf_mode=DoubleRow (2 MACs/cell/cycle,
contraction 256 per matmul). The projection weights are pre-scaled by 32 on
the host so their entries (~N(0, 1/1024)) sit in fp8e4's normal range; the
1/32 is folded into the tanh activation's scale. Both projections that
survive the softmax cancellation (W_i1, W_d2) are bias-free, so the scale
factors through cleanly. The attention-weighted sum over the raw features
(stage 2) stays bf16 for accuracy; softmax/normalization is fp32.
"""

import sys
import numpy as np
import ml_dtypes

_BF16 = ml_dtypes.bfloat16
_F8 = ml_dtypes.float8_e4m3fn

for _p in ("/opt/trn_rl_repo", "/root/.axon_site/_ro/trn_rl_repo"):
    if _p not in sys.path:
        sys.path.append(_p)

B, S, R, H = 32, 512, 196, 1024
NCORES = 8
BLOC = B // NCORES          # batches per core
OC = 512                    # output-chunk (one fp32 PSUM bank)
HC = H // 128               # contraction chunks of 128
HC2 = H // 256              # contraction chunks of 256 (fp8 DoubleRow)
USE_FP8 = False
W_SCALE = 32.0              # host pre-scale on fp8 weights; undone in tanh

_CACHE = {}


def _row_chunks(n):
    out, o = [], 0
    while o < n:
        out.append((o, min(128, n - o)))
        o += 128
    return out


def build_nc(use_fp8=USE_FP8):
    from concourse import bacc, mybir
    from concourse import tile

    f32, f16 = mybir.dt.float32, mybir.dt.bfloat16
    f8 = mybir.dt.float8e4
    xdt = f8 if use_fp8 else f16
    Act = mybir.ActivationFunctionType
    Alu = mybir.AluOpType
    DR = mybir.MatmulPerfMode.DoubleRow if use_fp8 else None

    nc = bacc.Bacc("TRN2", target_bir_lowering=False, debug=False)

    RP = 256  # img row count padded to a partition multiple for single-DMA loads
    xt_dns = nc.dram_tensor("xt_dns", [BLOC, HC, 128, S], xdt, kind="ExternalInput")
    xn_dns = nc.dram_tensor("xn_dns", [BLOC, S, H], f16, kind="ExternalInput")
    xt_img = nc.dram_tensor("xt_img", [BLOC, HC, 128, R], xdt, kind="ExternalInput")
    xn_img = nc.dram_tensor("xn_img", [BLOC, RP, H], f16, kind="ExternalInput")
    wt_i1 = nc.dram_tensor("wt_i1", [HC, 128, H], xdt, kind="ExternalInput")
    wt_d2 = nc.dram_tensor("wt_d2", [HC, 128, H], xdt, kind="ExternalInput")
    wrow_b = nc.dram_tensor("wrow_b", [128, H], f16, kind="ExternalInput")
    wrow_d = nc.dram_tensor("wrow_d", [128, H], f16, kind="ExternalInput")
    out_dns = nc.dram_tensor("out_dns", [BLOC, H], f32, kind="ExternalOutput")
    out_img = nc.dram_tensor("out_img", [BLOC, H], f32, kind="ExternalOutput")

    tanh_scale = (1.0 / W_SCALE) if use_fp8 else 1.0

    with tile.TileContext(nc) as tc:
        with (
            tc.tile_pool(name="const", bufs=1) as cpool,
            tc.tile_pool(name="xts", bufs=2) as xtpool,
            tc.tile_pool(name="xns", bufs=2) as xnpool,
            tc.tile_pool(name="work", bufs=3) as wpool,
            tc.tile_pool(name="small", bufs=12) as spool,
            tc.tile_pool(name="outs", bufs=2) as opool,
            tc.tile_pool(name="pp", bufs=3, space="PSUM") as ppool,
            tc.tile_pool(name="ps", bufs=2, space="PSUM") as pstat,
        ):
            # lazy const loads: weight DMAs are interleaved with the first
            # activation loads (per-chunk) at first use, so the first
            # projection group's dependencies land early in the queue
            wt_sb, wrow_sb = {}, {}

            def get_wt(nm):
                if nm not in wt_sb:
                    w = cpool.tile([128, HC * H], xdt, name=f"wt_{nm}_sb")
                    wt_sb[nm] = w
                return wt_sb[nm]

            def load_wt_chunk(nm, hc):
                dram = {"i1": wt_i1, "d2": wt_d2}[nm]
                w = wt_sb[nm]
                nc.sync.dma_start(out=w[:, hc * H:(hc + 1) * H], in_=dram[hc])

            def get_wrow(nm):
                if nm not in wrow_sb:
                    dram = {"b": wrow_b, "d": wrow_d}[nm]
                    w = cpool.tile([128, H], f16, name=f"wrow_{nm}_sb")
                    nc.sync.dma_start(out=w[:, :], in_=dram[:, :])
                    wrow_sb[nm] = w
                return wrow_sb[nm]

            ones_col = cpool.tile([128, 1], f16, name="ones_col")
            nc.vector.memset(ones_col[:, :], 1.0)

            for b in range(BLOC):
                for side in ("img", "dns"):
                    n_rows = R if side == "img" else S
                    xt_d = xt_img if side == "img" else xt_dns
                    xn_d = xn_img if side == "img" else xn_dns
                    wt_name = "i1" if side == "img" else "d2"
                    load_wt = wt_name not in wt_sb
                    wt = get_wt(wt_name)
                    out_d = out_img if side == "img" else out_dns
                    rcs = _row_chunks(n_rows)

                    # -- loads: on a weight's first use, interleave per-hc wt/xt
                    # chunks so the first projection group's deps land first;
                    # afterwards one 3D DMA covers the whole xt tile --
                    xt_t = xtpool.tile([128, HC * n_rows], xdt,
                                       name=f"xt_{side}_{b}", tag=f"xt_{side}")
                    if load_wt:
                        for hc in range(HC):
                            load_wt_chunk(wt_name, hc)
                            nc.sync.dma_start(
                                out=xt_t[:, hc * n_rows:(hc + 1) * n_rows],
                                in_=xt_d[b, hc])
                    else:
                        nc.sync.dma_start(
                            out=xt_t.rearrange("p (hc m) -> p hc m", hc=HC),
                            in_=xt_d[b].rearrange("hc p m -> p hc m"))
                    xt3 = xt_t.rearrange("p (hc m) -> p hc m", hc=HC)
                    wt3 = wt.rearrange("p (hc m) -> p hc m", hc=HC)

                    # -- projection, tanh, weighted o-reduction, exp --
                    # xn / wrow loads are issued after the first proj group so
                    # the projection's own dependencies lead the DMA queues
                    acols = []
                    xn_ts = []
                    wr = None
                    s_ps = pstat.tile([1, 1], f32, name=f"s_{side}_{b}", tag="stat")
                    for ci, (r0, rk) in enumerate(rcs):
                        ps = ppool.tile([128, H], f32, name=f"proj_{side}_{ci}_{b}",
                                        tag="pp")
                        if use_fp8:
                            for c in range(HC2):
                                lhs = xt3[:, 2 * c:2 * c + 2, r0:r0 + rk]
                                for oc in range(2):
                                    nc.tensor.matmul(
                                        ps[0:rk, oc * OC:(oc + 1) * OC],
                                        lhsT=lhs,
                                        rhs=wt3[:, 2 * c:2 * c + 2,
                                                oc * OC:(oc + 1) * OC],
                                        start=(c == 0), stop=(c == HC2 - 1),
                                        perf_mode=DR)
                        else:
                            for hc in range(HC):
                                lhs = xt3[:, hc, r0:r0 + rk]
                                for oc in range(2):
                                    nc.tensor.matmul(
                                        ps[0:rk, oc * OC:(oc + 1) * OC],
                                        lhsT=lhs,
                                        rhs=wt3[:, hc, oc * OC:(oc + 1) * OC],
                                        start=(hc == 0), stop=(hc == HC - 1))
                        if ci == 0:
                            nrc = len(rcs)
                            xn_t = xnpool.tile([128, nrc * H], f16,
                                               name=f"xn_{side}_{b}", tag=f"xn_{side}")
                            nc.sync.dma_start(
                                out=xn_t.rearrange("p (rc n) -> p rc n", rc=nrc),
                                in_=xn_d[b, 0:nrc * 128, :]
                                .rearrange("(rc p) n -> p rc n", p=128))
                            xn_ts = [xn_t[:, cj * H:(cj + 1) * H] for cj in range(nrc)]
                            wr = get_wrow("b" if side == "img" else "d")
                        th = wpool.tile([128, H], f16, name=f"th_{side}_{ci}_{b}", tag="th")
                        nc.scalar.activation(th[0:rk, :], ps[0:rk, :], Act.Tanh,
                                             scale=tanh_scale)
                        scr = wpool.tile([128, H], f16, name=f"scr_{side}_{ci}_{b}",
                                         tag="scr", bufs=2)
                        tcol = spool.tile([128, 1], f32, name=f"tc_{side}_{ci}_{b}", tag="tcol")
                        nc.vector.scalar_tensor_tensor(
                            out=scr[0:rk, :], in0=th[0:rk, :], scalar=1.0,
                            in1=wr[0:rk, :], op0=Alu.mult, op1=Alu.mult,
                            accum_out=tcol[0:rk, :])
                        acol = spool.tile([128, 1], f16, name=f"a_{side}_{ci}_{b}",
                                          tag=f"acol_{side}_{ci}")
                        nc.scalar.activation(acol[0:rk, :], tcol[0:rk, :], Act.Exp)
                        acols.append((acol, rk))
                        nc.tensor.matmul(
                            s_ps[0:1, 0:1], lhsT=acol[0:rk, 0:1], rhs=ones_col[0:rk, 0:1],
                            start=(ci == 0), stop=(ci == len(rcs) - 1))

                    # -- 1/sum, broadcast to 128 partitions (idle GPSIMD) --
                    r_sb = spool.tile([1, 1], f32, name=f"r_{side}_{b}", tag="r")
                    nc.vector.reciprocal(r_sb[0:1, 0:1], s_ps[0:1, 0:1])
                    rb_sb = spool.tile([128, 1], f32, name=f"rbs_{side}_{b}", tag="rb")
                    nc.gpsimd.partition_broadcast(rb_sb[:, 0:1], r_sb[0:1, 0:1])

                    # -- stage 2: out[h] = sum_r a_r x[r, h] (M=128 rows, all
                    # equal; only row 0 is written out) --
                    att_ps = ppool.tile([128, H], f32, name=f"att_{side}_{b}", tag="pp")
                    for h2 in range(2):
                        for ci, (r0, rk) in enumerate(rcs):
                            acol, _ = acols[ci]
                            nc.tensor.matmul(
                                att_ps[:, h2 * OC:(h2 + 1) * OC],
                                lhsT=acol[0:rk, 0:1].to_broadcast((rk, 128)),
                                rhs=xn_ts[ci][0:rk, h2 * OC:(h2 + 1) * OC],
                                start=(ci == 0), stop=(ci == len(rcs) - 1))
                    att_sb = opool.tile([128, H], f32, name=f"attsb_{side}_{b}",
                                        tag=f"att_{side}")
                    for h2 in range(2):
                        nc.scalar.activation(att_sb[:, h2 * OC:(h2 + 1) * OC],
                                             att_ps[:, h2 * OC:(h2 + 1) * OC],
                                             Act.Copy, scale=rb_sb[:, 0:1])
                    nc.sync.dma_start(out=out_d[b:b + 1, :], in_=att_sb[0:1, :])
    nc.compile()
    return nc


def _get_nc():
    if "nc" not in _CACHE:
        _CACHE["nc"] = build_nc()
    return _CACHE["nc"]


def make_in_maps(inputs):
    dns = np.ascontiguousarray(np.asarray(inputs["dns_feature"], dtype=np.float32))
    img = np.ascontiguousarray(np.asarray(inputs["img_features"], dtype=np.float32))
    W_i1 = np.asarray(inputs["W_i1"], dtype=np.float32)
    W_d2 = np.asarray(inputs["W_d2"], dtype=np.float32)
    wB = np.asarray(inputs["w_att1"], dtype=np.float32)[H:]
    wD = np.asarray(inputs["w_att2"], dtype=np.float32)[H:]

    xdt = _F8 if USE_FP8 else _BF16
    wsc = W_SCALE if USE_FP8 else 1.0
    wt_i1 = np.ascontiguousarray(W_i1.T * wsc).reshape(HC, 128, H).astype(xdt)
    wt_d2 = np.ascontiguousarray(W_d2.T * wsc).reshape(HC, 128, H).astype(xdt)
    wrow_b = np.ascontiguousarray(np.broadcast_to(wB, (128, H))).astype(_BF16)
    wrow_d = np.ascontiguousarray(np.broadcast_to(wD, (128, H))).astype(_BF16)

    xt_dns = np.ascontiguousarray(
        dns.transpose(0, 2, 1).reshape(B, HC, 128, S).astype(xdt))
    xt_img = np.ascontiguousarray(
        img.transpose(0, 2, 1).reshape(B, HC, 128, R).astype(xdt))
    xn_dns = dns.astype(_BF16)
    xn_img = np.zeros((B, 256, H), dtype=_BF16)
    xn_img[:, :R, :] = img.astype(_BF16)

    in_maps = []
    for k in range(NCORES):
        sl = slice(k * BLOC, (k + 1) * BLOC)
        in_maps.append({
            "xt_dns": np.ascontiguousarray(xt_dns[sl]),
            "xn_dns": np.ascontiguousarray(xn_dns[sl]),
            "xt_img": np.ascontiguousarray(xt_img[sl]),
            "xn_img": np.ascontiguousarray(xn_img[sl]),
            "wt_i1": wt_i1,
            "wt_d2": wt_d2,
            "wrow_b": wrow_b,
            "wrow_d": wrow_d,
        })
    return in_maps


def kernel(**inputs):
    from concourse.bass_utils import run_bass_kernel_spmd

    nc = _get_nc()
    in_maps = make_in_maps(inputs)
    res = run_bass_kernel_spmd(nc, in_maps, list(range(NCORES))).results
    att_dns = np.concatenate([res[k]["out_dns"] for k in range(NCORES)], axis=0)
    att_img = np.concatenate([res[k]["out_img"] for k in range(NCORES)], axis=0)
    out_dns = np.ascontiguousarray(
        np.broadcast_to(att_dns[:, None, :], (B, S, H)))
    out_img = np.ascontiguousarray(
        np.broadcast_to(att_img[:, None, :], (B, S, H)))
    return out_dns, out_img


# revision 8
# speedup vs baseline: 1.4065x; 1.1400x over previous
"""CoAttention ImageDNS kernel for Trainium2 (8 NeuronCores, Bass/Tile).

Math: the reference computes two additive-attention blocks. In both, the
softmax'd score is  score[b, q, k] = f(q-side)[b, q] + g(k-side)[b, k] + c,
and softmax over k is invariant to the q-dependent (and constant) terms, so
the attention weights are independent of the query index:

  visual_att[b, s, :]  = softmax_r( wB . tanh(W_i1 @ img[b, r]) )
  textual_att[b, i, :] = softmax_j( wD . tanh(W_d2 @ dns[b, j]) )

Hence both outputs are per-batch rank-1 broadcasts:

  att_img_features[b, s, :] = visual_att[b]  @ img[b]   (same for all s)
  att_dns_features[b, i, :] = textual_att[b] @ dns[b]   (same for all i)

W_d1/b_d1/w_att1[:H]/b_att1/W_i2/b_i2/w_att2[:H]/b_att2 cancel entirely.

Sharding: pure data-parallel over batch, 4 batches per core, no collectives.
The device computes the per-batch [H] attention outputs; the host broadcasts
them over the (identical) S query rows, so the kernel writes only B*H values
instead of B*S*H.

Engine split (keeps the PE stream pure projection matmuls):
  - PE:     projections over row-packed activations (all 4 batches' rows
            concatenated: img 784 rows -> 7 chunks, dns 2048 -> 16), plus
            one tiny score-column transpose per score group.
  - Scalar: tanh (bf16 out), exp, per-batch softmax sums via Copy+accum.
  - Vector: weighted o-reduction for scores (stt), stage-2 weighted row
            sums over the transposed activations (stt accum), reciprocals.
  - GpSimd: partition-broadcast of the attention-weight rows.
Stage 2 consumes the same transposed activation tiles as the projections,
so the natural-layout activations are never loaded: HBM in is ~10MB/core.
"""

import sys
import numpy as np
import ml_dtypes

_BF16 = ml_dtypes.bfloat16

for _p in ("/opt/trn_rl_repo", "/root/.axon_site/_ro/trn_rl_repo"):
    if _p not in sys.path:
        sys.path.append(_p)

B, S, R, H = 32, 512, 196, 1024
NCORES = 8
BLOC = B // NCORES          # batches per core
OC = 512                    # output-chunk (one fp32 PSUM bank)
HC = H // 128               # contraction chunks of 128
NI = BLOC * R               # img rows, all batches packed (784)
ND = BLOC * S               # dns rows, all batches packed (2048)

_CACHE = {}


def _row_chunks(n):
    out, o = [], 0
    while o < n:
        out.append((o, min(128, n - o)))
        o += 128
    return out


def build_nc():
    from concourse import bacc, mybir
    from concourse import tile

    f32, f16 = mybir.dt.float32, mybir.dt.bfloat16
    Act = mybir.ActivationFunctionType
    Alu = mybir.AluOpType

    nc = bacc.Bacc("TRN2", target_bir_lowering=False, debug=False)

    xt_img = nc.dram_tensor("xt_img", [HC, 128, NI], f16, kind="ExternalInput")
    xt_dns = nc.dram_tensor("xt_dns", [HC, 128, ND], f16, kind="ExternalInput")
    wt_i1 = nc.dram_tensor("wt_i1", [HC, 128, H], f16, kind="ExternalInput")
    wt_d2 = nc.dram_tensor("wt_d2", [HC, 128, H], f16, kind="ExternalInput")
    wrow_b = nc.dram_tensor("wrow_b", [128, H], f16, kind="ExternalInput")
    wrow_d = nc.dram_tensor("wrow_d", [128, H], f16, kind="ExternalInput")
    ident_d = nc.dram_tensor("ident", [128, 128], f16, kind="ExternalInput")
    # [p, b, hc] layout: element h of batch b lives at [h % 128, b, h // 128]
    out_dns = nc.dram_tensor("out_dns", [128, BLOC, HC], f32, kind="ExternalOutput")
    out_img = nc.dram_tensor("out_img", [128, BLOC, HC], f32, kind="ExternalOutput")

    with tile.TileContext(nc) as tc:
        with (
            tc.tile_pool(name="const", bufs=1) as cpool,
            tc.tile_pool(name="work", bufs=3) as wpool,
            tc.tile_pool(name="small", bufs=8) as spool,
            tc.tile_pool(name="pp", bufs=2, space="PSUM") as ppool,
            tc.tile_pool(name="pt", bufs=1, space="PSUM") as ptps,
        ):
            wt_sb = {}

            def load_wt_chunk(nm, hc):
                dram = {"i1": wt_i1, "d2": wt_d2}[nm]
                nc.sync.dma_start(out=wt_sb[nm][:, hc * H:(hc + 1) * H], in_=dram[hc])

            ident = cpool.tile([128, 128], f16, name="ident_sb")
            nc.sync.dma_start(out=ident[:, :], in_=ident_d[:, :])

            # ---- activation / weight tiles (whole side packed) ----
            xt_i = cpool.tile([128, HC * NI], f16, name="xt_img_sb")
            xt_d = cpool.tile([128, HC * ND], f16, name="xt_dns_sb")
            wt_sb["i1"] = cpool.tile([128, HC * H], f16, name="wt_i1_sb")
            wt_sb["d2"] = cpool.tile([128, HC * H], f16, name="wt_d2_sb")
            wr_b = cpool.tile([128, H], f16, name="wrow_b_sb")
            wr_d = cpool.tile([128, H], f16, name="wrow_d_sb")

            # img-side loads first (small first deps -> PE starts early):
            # interleave wt_i1/xt_img per-hc so proj group 0 can begin after
            # the first pair lands
            for hc in range(HC):
                load_wt_chunk("i1", hc)
                nc.sync.dma_start(out=xt_i[:, hc * NI:(hc + 1) * NI],
                                  in_=xt_img[hc])
            nc.sync.dma_start(out=wr_b[:, :], in_=wrow_b[:, :])
            # dns loads follow in queue order; they complete long before use
            nc.sync.dma_start(
                out=xt_d.rearrange("p (hc m) -> p hc m", hc=HC),
                in_=xt_dns.rearrange("hc p m -> p hc m"))
            for hc in range(HC):
                load_wt_chunk("d2", hc)
            nc.sync.dma_start(out=wr_d[:, :], in_=wrow_d[:, :])

            xt3 = {"img": xt_i.rearrange("p (hc m) -> p hc m", hc=HC),
                   "dns": xt_d.rearrange("p (hc m) -> p hc m", hc=HC)}
            wt3 = {"img": wt_sb["i1"].rearrange("p (hc m) -> p hc m", hc=HC),
                   "dns": wt_sb["d2"].rearrange("p (hc m) -> p hc m", hc=HC)}
            wr = {"img": wr_b, "dns": wr_d}
            n_rows = {"img": R, "dns": S}
            out_d = {"img": out_img, "dns": out_dns}

            # score groups: (side, row range, batches whose tails it owns)
            # img: one group (all 4 batches, 784 rows -> 7 chunks)
            # dns: 4 groups of one batch each (512 rows -> 4 chunks)
            groups = [("img", 0, NI, [0, 1, 2, 3])] + \
                     [("dns", b * S, (b + 1) * S, [b]) for b in range(BLOC)]

            att_sb = {s: cpool.tile([128, BLOC * HC], f32, name=f"att_{s}_sb")
                      for s in ("img", "dns")}

            def emit_proj_scores(gi):
                """PE proj + tanh + score stt + exp for group gi.
                Returns state for the deferred tail."""
                side, g0, g1, _ = groups[gi]
                rcs = _row_chunks(g1 - g0)
                acs = spool.tile([128, len(rcs)], f16, name=f"acs_{gi}",
                                 tag="acs", bufs=2)
                for ci, (c0, rk) in enumerate(rcs):
                    r0 = g0 + c0
                    ps = ppool.tile([128, H], f32, name=f"proj_{gi}_{ci}",
                                    tag="pp")
                    for hc in range(HC):
                        lhs = xt3[side][:, hc, r0:r0 + rk]
                        for oc in range(2):
                            nc.tensor.matmul(
                                ps[0:rk, oc * OC:(oc + 1) * OC],
                                lhsT=lhs,
                                rhs=wt3[side][:, hc, oc * OC:(oc + 1) * OC],
                                start=(hc == 0), stop=(hc == HC - 1))
                    th = wpool.tile([128, H], f16, name=f"th_{gi}_{ci}", tag="th")
                    nc.scalar.activation(th[0:rk, :], ps[0:rk, :], Act.Tanh)
                    scr = wpool.tile([128, H], f16, name=f"scr_{gi}_{ci}",
                                     tag="scr", bufs=2)
                    tcol = spool.tile([128, 1], f32, name=f"tc_{gi}_{ci}",
                                      tag="tcol", bufs=4)
                    nc.vector.scalar_tensor_tensor(
                        out=scr[0:rk, :], in0=th[0:rk, :], scalar=1.0,
                        in1=wr[side][0:rk, :], op0=Alu.mult, op1=Alu.mult,
                        accum_out=tcol[0:rk, :])
                    nc.scalar.activation(acs[0:rk, ci:ci + 1], tcol[0:rk, :],
                                         Act.Exp)
                return acs, rcs

            def emit_tail(gi, acs, rcs):
                """Broadcast each score-column chunk to [128, rk] rows via a
                tiny PE matmul (a-col x identity), then per-batch softmax
                sums + stage-2 weighted row sums (vector), final scale."""
                side, g0, g1, batches = groups[gi]
                ab_ps = ptps.tile([128, g1 - g0], f32, name=f"abps_{gi}",
                                  tag=f"abps_{side}",
                                  bufs=1 if side == "img" else 2)
                for ci, (c0, rk) in enumerate(rcs):
                    nc.tensor.matmul(
                        ab_ps[:, c0:c0 + rk],
                        lhsT=acs[0:rk, ci:ci + 1].to_broadcast((rk, 128)),
                        rhs=ident[0:rk, 0:rk], start=True, stop=True)
                a_b = wpool.tile([128, g1 - g0], f16, name=f"ab_{gi}",
                                 tag=f"ab_{side}", bufs=1 if side == "img" else 2)
                nc.vector.tensor_copy(a_b[:, :], ab_ps[:, :])
                for b in batches:
                    o0 = b * n_rows[side] - g0
                    nr = n_rows[side]
                    scrap = wpool.tile([128, S], f16, name=f"scrap_{gi}_{b}",
                                       tag="scrap", bufs=2)
                    asum = spool.tile([128, 1], f32, name=f"asum_{gi}_{b}",
                                      tag="asum", bufs=2)
                    nc.scalar.activation(scrap[:, 0:nr], a_b[:, o0:o0 + nr],
                                         Act.Copy, accum_out=asum[:, 0:1])
                    rb = spool.tile([128, 1], f32, name=f"rb_{gi}_{b}",
                                    tag="rb", bufs=2)
                    nc.vector.reciprocal(rb[:, 0:1], asum[:, 0:1])
                    attc = spool.tile([128, HC], f32, name=f"attc_{gi}_{b}",
                                      tag="attc", bufs=2)
                    scr2 = wpool.tile([128, S], f16, name=f"sc2_{gi}_{b}",
                                      tag="scr2", bufs=2)
                    for hc in range(HC):
                        nc.vector.scalar_tensor_tensor(
                            out=scr2[:, 0:nr],
                            in0=xt3[side][:, hc, b * nr:(b + 1) * nr],
                            scalar=1.0, in1=a_b[:, o0:o0 + nr],
                            op0=Alu.mult, op1=Alu.mult,
                            accum_out=attc[:, hc:hc + 1])
                    nc.scalar.activation(
                        att_sb[side][:, b * HC:(b + 1) * HC], attc[:, 0:HC],
                        Act.Copy, scale=rb[:, 0:1])

            # pipeline: group i's projections, then group i-1's tail
            state = {}
            for gi in range(len(groups)):
                state[gi] = emit_proj_scores(gi)
                if gi - 1 in state:
                    emit_tail(gi - 1, *state[gi - 1])
            emit_tail(len(groups) - 1, *state[len(groups) - 1])

            for side in ("img", "dns"):
                nc.sync.dma_start(
                    out=out_d[side].rearrange("p b hc -> p (b hc)"),
                    in_=att_sb[side][:, :])
    nc.compile()
    return nc


def _get_nc():
    if "nc" not in _CACHE:
        _CACHE["nc"] = build_nc()
    return _CACHE["nc"]


def make_in_maps(inputs):
    dns = np.ascontiguousarray(np.asarray(inputs["dns_feature"], dtype=np.float32))
    img = np.ascontiguousarray(np.asarray(inputs["img_features"], dtype=np.float32))
    W_i1 = np.asarray(inputs["W_i1"], dtype=np.float32)
    W_d2 = np.asarray(inputs["W_d2"], dtype=np.float32)
    wB = np.asarray(inputs["w_att1"], dtype=np.float32)[H:]
    wD = np.asarray(inputs["w_att2"], dtype=np.float32)[H:]

    wt_i1 = np.ascontiguousarray(W_i1.T).reshape(HC, 128, H).astype(_BF16)
    wt_d2 = np.ascontiguousarray(W_d2.T).reshape(HC, 128, H).astype(_BF16)
    wrow_b = np.ascontiguousarray(np.broadcast_to(wB, (128, H))).astype(_BF16)
    wrow_d = np.ascontiguousarray(np.broadcast_to(wD, (128, H))).astype(_BF16)
    ident = np.eye(128, dtype=_BF16)

    in_maps = []
    for k in range(NCORES):
        sl = slice(k * BLOC, (k + 1) * BLOC)
        # [BLOC, rows, H] -> [H, BLOC*rows] -> [HC, 128, n]
        xt_d = dns[sl].reshape(BLOC * S, H).T.reshape(HC, 128, BLOC * S)
        xt_i = img[sl].reshape(BLOC * R, H).T.reshape(HC, 128, BLOC * R)
        in_maps.append({
            "xt_dns": np.ascontiguousarray(xt_d).astype(_BF16),
            "xt_img": np.ascontiguousarray(xt_i).astype(_BF16),
            "wt_i1": wt_i1,
            "wt_d2": wt_d2,
            "wrow_b": wrow_b,
            "wrow_d": wrow_d,
            "ident": ident,
        })
    return in_maps


def kernel(**inputs):
    from concourse.bass_utils import run_bass_kernel_spmd

    nc = _get_nc()
    in_maps = make_in_maps(inputs)
    res = run_bass_kernel_spmd(nc, in_maps, list(range(NCORES))).results
    # device out: [128, BLOC, HC], element h of batch b at [h % 128, b, h//128]
    outs = {}
    for name in ("out_dns", "out_img"):
        per = [res[k][name].transpose(1, 2, 0).reshape(BLOC, H)
               for k in range(NCORES)]
        outs[name] = np.concatenate(per, axis=0)
    out_dns = np.ascontiguousarray(
        np.broadcast_to(outs["out_dns"][:, None, :], (B, S, H)))
    out_img = np.ascontiguousarray(
        np.broadcast_to(outs["out_img"][:, None, :], (B, S, H)))
    return out_dns, out_img
